# revision 2
# baseline (speedup 1.0000x reference)
"""DETR scene-graph predicate head on 8 Trainium2 NeuronCores.

Math: logits[l,b,r,:] = concat(hs[l,b,q_sub], hs[l,b,q_obj]) @ W_pred.T + b_pred
where q_sub/q_obj come from (tgt_perm inverse, relationships, src_indices) —
pure integer index math, done on host.

Key structure: relations only reference matched query slots, so only the
distinct queries actually used per (layer,image) block matter (~43 of 101 on
average).  The concat-linear decomposes per relation:
  logits[r,p] = A[q_sub(r),p] + B[q_obj(r),p] + b,  A = hs@W1.T, B = hs@W2.T
so the device computes the stacked A|B table (102 live channels) over the
ragged stream of used (block, query) slots with dense matmuls; the host does
the final O(L*B*R*P) index-select + add + bias.

Device layout (batch axis sharded 8 ways; L*B/8 = 192 blocks/core):
  - The per-core slot stream is fully ragged — no per-block padding.  Host
    concatenates each block's distinct-query hs columns into S_pad slots
    (padded only to the max core count, mult of 32) and splits the stream
    into T tiles of 512 slots (last ragged).  Block boundaries are irrelevant
    on device: every output column depends only on its own input column.
  - Per tile: hst cols [chunk0(W) | chunk1(W)] bf16 (d on partitions,
    2 chunks of 128 for the D=256 contraction), one ~256KB HWDGE load on the
    sync ring, 2 accumulating matmuls into one psum bank [128, W] f32 with
    the stationary wpk operand, one DVE/ACT cast (alternating) of the 102
    live rows to bf16, and one ~200KB store per tile pair on the scalar ring.
  - wpk [128, (chunk, 128)]: cols 0:51 W1.T, 51:102 W2.T, rest zero.

Streaming ~6MB/core through the 16 SDMA engines at ~350GB/s is the roofline;
compute (matmul ~7us, casts split across DVE+ACT) hides under it.
"""

import sys

import numpy as np

L, B, Q1, D = 6, 256, 101, 256
M, R, P = 64, 64, 51
NCORES = 8
BLOC = B // NCORES          # images per core
NB = L * BLOC               # (layer, image) blocks per core
TW = 512                    # slots per device tile (one psum bank)
P2 = 2 * P                  # 102 live logit channels (sub | obj halves)

_CACHE = {}


def _build_program(S_pad):
    import concourse.bacc as bacc
    import concourse.mybir as mybir
    import concourse.tile as tile
    from contextlib import ExitStack

    f32 = mybir.dt.float32
    bf16 = mybir.dt.bfloat16
    nc = bacc.Bacc("TRN2", target_bir_lowering=False, debug=False)

    T = -(-S_pad // TW)
    hst = nc.dram_tensor("hst", [128, 2 * S_pad], bf16, kind="ExternalInput").ap()
    wpk = nc.dram_tensor("wpk", [128, 256], bf16, kind="ExternalInput").ap()
    outab = nc.dram_tensor("outab", [P2, S_pad], bf16, kind="ExternalOutput").ap()

    with tile.TileContext(nc) as tc, ExitStack() as ctx:
        const = ctx.enter_context(tc.tile_pool(name="const", bufs=1))
        inp = ctx.enter_context(tc.tile_pool(name="inp", bufs=6))
        outp = ctx.enter_context(tc.tile_pool(name="outp", bufs=3))
        psA = ctx.enter_context(tc.tile_pool(name="psA", bufs=6, space="PSUM"))

        wpk_t = const.tile([128, 256], bf16)
        nc.scalar.dma_start(out=wpk_t[:], in_=wpk[:])

        cast_flip = 0
        o_t = None
        for t in range(T):
            W = min(TW, S_pad - t * TW)
            in_t = inp.tile([128, 2 * TW], bf16, tag="h")
            nc.sync.dma_start(out=in_t[:, 0:2 * W],
                              in_=hst[:, 2 * TW * t:2 * TW * t + 2 * W])
            if t % 2 == 0:
                o_t = outp.tile([P2, 2 * TW], bf16, tag="o")
            half = (t % 2) * TW

            ps = psA.tile([128, TW], f32, tag="ps")
            nc.tensor.matmul(out=ps[:, 0:W], lhsT=wpk_t[:, 0:128],
                             rhs=in_t[:, 0:W], start=True, stop=False)
            nc.tensor.matmul(out=ps[:, 0:W], lhsT=wpk_t[:, 128:256],
                             rhs=in_t[:, W:2 * W], start=False, stop=True)
            if cast_flip == 0:
                nc.vector.tensor_copy(out=o_t[:, half:half + W],
                                      in_=ps[0:P2, 0:W])
            else:
                nc.scalar.copy(out=o_t[:, half:half + W], in_=ps[0:P2, 0:W])
            cast_flip ^= 1

            if t % 2 == 1 or t == T - 1:
                p0 = (t // 2) * 2 * TW
                n = min(2 * TW, S_pad - p0)
                nc.scalar.dma_start(out=outab[:, p0:p0 + n], in_=o_t[:, 0:n])

    nc.compile()
    return nc


def _host_indices(src_indices, tgt_perm, relationships):
    """q_sub, q_obj: [L, B, R] int64 — query slot per relation."""
    src = np.asarray(src_indices, dtype=np.int64)
    tgt = np.asarray(tgt_perm, dtype=np.int64)
    rel = np.asarray(relationships, dtype=np.int64)

    # lookup[l, b, tgt[l, b, k]] = k
    lookup = np.empty((L, B, M), dtype=np.int64)
    li = np.arange(L)[:, None, None]
    bi = np.arange(B)[None, :, None]
    lookup[li, bi, tgt] = np.broadcast_to(np.arange(M), (L, B, M))

    sub_t = np.broadcast_to(rel[None, :, :, 0], (L, B, R))
    obj_t = np.broadcast_to(rel[None, :, :, 1], (L, B, R))
    pos_sub = np.take_along_axis(lookup, sub_t, axis=2)
    pos_obj = np.take_along_axis(lookup, obj_t, axis=2)
    q_sub = np.take_along_axis(src, pos_sub, axis=2)
    q_obj = np.take_along_axis(src, pos_obj, axis=2)
    return q_sub, q_obj


def _host_prepare(hs, src_indices, tgt_perm, relationships, W_pred, b_pred):
    """Ragged distinct-query compaction + device input packing."""
    import ml_dtypes
    bf16 = ml_dtypes.bfloat16

    hs = np.asarray(hs, dtype=np.float32)
    W = np.asarray(W_pred, dtype=np.float32)

    q_sub, q_obj = _host_indices(src_indices, tgt_perm, relationships)

    used = np.zeros((L * B, Q1), dtype=bool)
    rows = np.arange(L * B)[:, None]
    qcat = np.concatenate([q_sub, q_obj], axis=-1).reshape(L * B, 2 * R)
    used[rows, qcat] = True
    nuniq = used.sum(axis=1).reshape(L, B)
    # stable argsort of ~used: first nuniq entries = used queries, ascending
    order = np.argsort(~used, axis=1, kind="stable").reshape(L, B, Q1)
    slot_map = (np.cumsum(used, axis=1) - 1).reshape(L, B, Q1)
    j_sub = np.take_along_axis(slot_map, q_sub, axis=2)   # [L, B, R]
    j_obj = np.take_along_axis(slot_map, q_obj, axis=2)

    S_c = [int(nuniq[:, c * BLOC:(c + 1) * BLOC].sum()) for c in range(NCORES)]
    S_pad = -(-max(S_c) // 32) * 32
    T = -(-S_pad // TW)

    # wpk [128, (chunk, 128)]: chunk k cols = Wpad[128k:128k+128, :]
    wpad = np.zeros((D, 128), dtype=np.float32)
    wpad[:, :P] = W[:, :D].T
    wpad[:, P:P2] = W[:, D:].T
    wpk = np.ascontiguousarray(
        wpad.reshape(2, 128, 128).transpose(1, 0, 2).reshape(128, 256)
    ).astype(bf16)

    hs_bf = hs.astype(bf16)
    in_maps = []
    offs = []
    for c in range(NCORES):
        sl = slice(c * BLOC, (c + 1) * BLOC)
        n = nuniq[:, sl]                               # [L, BLOC]
        nf = n.reshape(-1)
        off = np.concatenate([[0], np.cumsum(nf)[:-1]]).reshape(L, BLOC)
        offs.append(off)

        mask = np.arange(Q1)[None, :] < nf[:, None]    # [NB, Q1]
        q_of = order[:, sl].reshape(NB, Q1)[mask]      # [S] ragged concat
        l_of = np.repeat(np.repeat(np.arange(L), BLOC), nf)
        b_of = np.repeat(np.tile(np.arange(c * BLOC, (c + 1) * BLOC), L), nf)
        pad = S_pad - q_of.shape[0]
        q_of = np.concatenate([q_of, np.zeros(pad, dtype=q_of.dtype)])
        l_of = np.concatenate([l_of, np.zeros(pad, dtype=l_of.dtype)])
        b_of = np.concatenate([b_of, np.zeros(pad, dtype=b_of.dtype)])

        G = hs_bf[l_of, b_of, q_of]                    # [S_pad, 256]
        Tm1 = T - 1
        main = (G[:TW * Tm1].reshape(Tm1, TW, 256).transpose(0, 2, 1)
                .reshape(Tm1, 2, 128, TW).transpose(2, 0, 1, 3)
                .reshape(128, Tm1 * 2 * TW))
        Wl = S_pad - TW * Tm1
        last = (G[TW * Tm1:].T.reshape(2, 128, Wl).transpose(1, 0, 2)
                .reshape(128, 2 * Wl))
        hst = np.ascontiguousarray(np.concatenate([main, last], axis=1))
        in_maps.append({"hst": hst, "wpk": wpk})
    return S_pad, in_maps, j_sub, j_obj, offs


def kernel(hs, src_indices, tgt_perm, relationships, W_pred, b_pred):
    if "concourse" not in sys.modules:
        try:
            import concourse  # noqa: F401
        except ImportError:
            sys.path.insert(0, "/opt/trn_rl_repo")
    from concourse import bass_utils

    S_pad, in_maps, j_sub, j_obj, offs = _host_prepare(
        hs, src_indices, tgt_perm, relationships, W_pred, b_pred)
    if _CACHE.get("S_pad") != S_pad:
        _CACHE["nc"] = _build_program(S_pad)
        _CACHE["S_pad"] = S_pad
    nc = _CACHE["nc"]

    res = bass_utils.run_bass_kernel_spmd(nc, in_maps, list(range(NCORES)))

    b = np.asarray(b_pred, dtype=np.float32)
    outs = []
    for c in range(NCORES):
        ab = res.results[c]["outab"].astype(np.float32)   # [102, S_pad]
        sl = slice(c * BLOC, (c + 1) * BLOC)
        col_sub = offs[c][:, :, None] + j_sub[:, sl]      # [L, BLOC, R]
        col_obj = offs[c][:, :, None] + j_obj[:, sl]
        logits = (ab[:P, col_sub] + ab[P:P2, col_obj])    # [P, L, BLOC, R]
        outs.append(np.ascontiguousarray(logits.transpose(1, 2, 3, 0) + b))
    return np.concatenate(outs, axis=1)


# revision 4
# speedup vs baseline: 1.1264x; 1.1264x over previous
"""DETR scene-graph predicate head on 8 Trainium2 NeuronCores.

Math: logits[l,b,r,:] = concat(hs[l,b,q_sub], hs[l,b,q_obj]) @ W_pred.T + b_pred
where q_sub/q_obj come from (tgt_perm inverse, relationships, src_indices) —
pure integer index math, done on host.

Key structure: relations only reference matched query slots, so only the
distinct queries actually used per (layer,image) block matter (~43 of 101 on
average).  The concat-linear decomposes per relation:
  logits[r,p] = A[q_sub(r),p] + B[q_obj(r),p] + b,  A = hs@W1.T, B = hs@W2.T
so the device computes the stacked A|B table (102 live channels) over the
ragged stream of used (block, query) slots with dense matmuls; the host does
the final O(L*B*R*P) index-select + add + bias.

Device layout (batch axis sharded 8 ways; L*B/8 = 192 blocks/core):
  - The per-core slot stream is fully ragged — no per-block padding.  Host
    concatenates each block's distinct-query hs columns into S_pad slots
    (padded only to the max core count, mult of 32) and splits the stream
    into T tiles of 512 slots (last ragged).  Block boundaries are irrelevant
    on device: every output column depends only on its own input column.
  - Per tile: hst cols [chunk0(W) | chunk1(W)] bf16 (d on partitions,
    2 chunks of 128 for the D=256 contraction), one ~256KB HWDGE load on the
    sync ring, 2 accumulating matmuls into one psum bank [128, W] f32 with
    the stationary wpk operand, one DVE/ACT cast (alternating) of the 102
    live rows to bf16, and one ~200KB store per tile pair on the scalar ring.
  - wpk [128, (chunk, 128)]: cols 0:51 W1.T, 51:102 W2.T, rest zero.

Streaming ~6MB/core through the 16 SDMA engines at ~350GB/s is the roofline;
compute (matmul ~7us, casts split across DVE+ACT) hides under it.
"""

import sys

import numpy as np

L, B, Q1, D = 6, 256, 101, 256
M, R, P = 64, 64, 51
NCORES = 8
BLOC = B // NCORES          # images per core
NB = L * BLOC               # (layer, image) blocks per core
TW = 512                    # slots per device tile (one psum bank)
P2 = 2 * P                  # 102 live logit channels (sub | obj halves)

_CACHE = {}


def _build_program(S_pad):
    import concourse.bacc as bacc
    import concourse.mybir as mybir
    import concourse.tile as tile
    from contextlib import ExitStack

    f32 = mybir.dt.float32
    bf16 = mybir.dt.bfloat16
    nc = bacc.Bacc("TRN2", target_bir_lowering=False, debug=False)

    T = -(-S_pad // TW)
    hst = nc.dram_tensor("hst", [128, 2 * S_pad], bf16, kind="ExternalInput").ap()
    wpk = nc.dram_tensor("wpk", [128, 256], bf16, kind="ExternalInput").ap()
    # 128 store rows (102 live + 26 zero): partition-balanced across the 16
    # SDMA engines — a 102-partition store lands on a subset of engines only.
    outab = nc.dram_tensor("outab", [128, S_pad], bf16, kind="ExternalOutput").ap()

    with tile.TileContext(nc) as tc, ExitStack() as ctx:
        const = ctx.enter_context(tc.tile_pool(name="const", bufs=1))
        inp = ctx.enter_context(tc.tile_pool(name="inp", bufs=4))
        outp = ctx.enter_context(tc.tile_pool(name="outp", bufs=3))
        psA = ctx.enter_context(tc.tile_pool(name="psA", bufs=6, space="PSUM"))

        wpk_t = const.tile([128, 256], bf16)
        nc.scalar.dma_start(out=wpk_t[:], in_=wpk[:])

        cast_flip = 0
        for p0 in range(0, T, 2):              # pair of tiles per DMA/store
            tws = [min(TW, S_pad - t * TW) for t in (p0, p0 + 1) if t < T]
            cols = 2 * sum(tws)
            in_t = inp.tile([128, 4 * TW], bf16, tag="h")
            nc.sync.dma_start(out=in_t[:, 0:cols],
                              in_=hst[:, 2 * TW * p0:2 * TW * p0 + cols])
            o_t = outp.tile([128, 2 * TW], bf16, tag="o")

            for j, W in enumerate(tws):
                lo = j * 2 * TW
                ps = psA.tile([128, TW], f32, tag="ps")
                nc.tensor.matmul(out=ps[:, 0:W], lhsT=wpk_t[:, 0:128],
                                 rhs=in_t[:, lo:lo + W], start=True, stop=False)
                nc.tensor.matmul(out=ps[:, 0:W], lhsT=wpk_t[:, 128:256],
                                 rhs=in_t[:, lo + W:lo + 2 * W],
                                 start=False, stop=True)
                if cast_flip == 0:
                    nc.vector.tensor_copy(out=o_t[:, j * TW:j * TW + W],
                                          in_=ps[:, 0:W])
                else:
                    nc.scalar.copy(out=o_t[:, j * TW:j * TW + W],
                                   in_=ps[:, 0:W])
                cast_flip ^= 1

            n = sum(tws) if len(tws) == 1 else TW + tws[1]
            nc.scalar.dma_start(out=outab[:, TW * p0:TW * p0 + n],
                                in_=o_t[:, 0:n])

    nc.compile()
    return nc


def _host_indices(src_indices, tgt_perm, relationships):
    """q_sub, q_obj: [L, B, R] int64 — query slot per relation."""
    src = np.asarray(src_indices, dtype=np.int64)
    tgt = np.asarray(tgt_perm, dtype=np.int64)
    rel = np.asarray(relationships, dtype=np.int64)

    # lookup[l, b, tgt[l, b, k]] = k
    lookup = np.empty((L, B, M), dtype=np.int64)
    li = np.arange(L)[:, None, None]
    bi = np.arange(B)[None, :, None]
    lookup[li, bi, tgt] = np.broadcast_to(np.arange(M), (L, B, M))

    sub_t = np.broadcast_to(rel[None, :, :, 0], (L, B, R))
    obj_t = np.broadcast_to(rel[None, :, :, 1], (L, B, R))
    pos_sub = np.take_along_axis(lookup, sub_t, axis=2)
    pos_obj = np.take_along_axis(lookup, obj_t, axis=2)
    q_sub = np.take_along_axis(src, pos_sub, axis=2)
    q_obj = np.take_along_axis(src, pos_obj, axis=2)
    return q_sub, q_obj


def _host_prepare(hs, src_indices, tgt_perm, relationships, W_pred, b_pred):
    """Ragged distinct-query compaction + device input packing."""
    import ml_dtypes
    bf16 = ml_dtypes.bfloat16

    hs = np.asarray(hs, dtype=np.float32)
    W = np.asarray(W_pred, dtype=np.float32)

    q_sub, q_obj = _host_indices(src_indices, tgt_perm, relationships)

    used = np.zeros((L * B, Q1), dtype=bool)
    rows = np.arange(L * B)[:, None]
    qcat = np.concatenate([q_sub, q_obj], axis=-1).reshape(L * B, 2 * R)
    used[rows, qcat] = True
    nuniq = used.sum(axis=1).reshape(L, B)
    # stable argsort of ~used: first nuniq entries = used queries, ascending
    order = np.argsort(~used, axis=1, kind="stable").reshape(L, B, Q1)
    slot_map = (np.cumsum(used, axis=1) - 1).reshape(L, B, Q1)
    j_sub = np.take_along_axis(slot_map, q_sub, axis=2)   # [L, B, R]
    j_obj = np.take_along_axis(slot_map, q_obj, axis=2)

    S_c = [int(nuniq[:, c * BLOC:(c + 1) * BLOC].sum()) for c in range(NCORES)]
    S_pad = -(-max(S_c) // 32) * 32
    T = -(-S_pad // TW)

    # wpk [128, (chunk, 128)]: chunk k cols = Wpad[128k:128k+128, :]
    wpad = np.zeros((D, 128), dtype=np.float32)
    wpad[:, :P] = W[:, :D].T
    wpad[:, P:P2] = W[:, D:].T
    wpk = np.ascontiguousarray(
        wpad.reshape(2, 128, 128).transpose(1, 0, 2).reshape(128, 256)
    ).astype(bf16)

    hs_bf = hs.astype(bf16)
    in_maps = []
    offs = []
    for c in range(NCORES):
        sl = slice(c * BLOC, (c + 1) * BLOC)
        n = nuniq[:, sl]                               # [L, BLOC]
        nf = n.reshape(-1)
        off = np.concatenate([[0], np.cumsum(nf)[:-1]]).reshape(L, BLOC)
        offs.append(off)

        mask = np.arange(Q1)[None, :] < nf[:, None]    # [NB, Q1]
        q_of = order[:, sl].reshape(NB, Q1)[mask]      # [S] ragged concat
        l_of = np.repeat(np.repeat(np.arange(L), BLOC), nf)
        b_of = np.repeat(np.tile(np.arange(c * BLOC, (c + 1) * BLOC), L), nf)
        pad = S_pad - q_of.shape[0]
        q_of = np.concatenate([q_of, np.zeros(pad, dtype=q_of.dtype)])
        l_of = np.concatenate([l_of, np.zeros(pad, dtype=l_of.dtype)])
        b_of = np.concatenate([b_of, np.zeros(pad, dtype=b_of.dtype)])

        G = hs_bf[l_of, b_of, q_of]                    # [S_pad, 256]
        Tm1 = T - 1
        main = (G[:TW * Tm1].reshape(Tm1, TW, 256).transpose(0, 2, 1)
                .reshape(Tm1, 2, 128, TW).transpose(2, 0, 1, 3)
                .reshape(128, Tm1 * 2 * TW))
        Wl = S_pad - TW * Tm1
        last = (G[TW * Tm1:].T.reshape(2, 128, Wl).transpose(1, 0, 2)
                .reshape(128, 2 * Wl))
        hst = np.ascontiguousarray(np.concatenate([main, last], axis=1))
        in_maps.append({"hst": hst, "wpk": wpk})
    return S_pad, in_maps, j_sub, j_obj, offs


def kernel(hs, src_indices, tgt_perm, relationships, W_pred, b_pred):
    if "concourse" not in sys.modules:
        try:
            import concourse  # noqa: F401
        except ImportError:
            sys.path.insert(0, "/opt/trn_rl_repo")
    from concourse import bass_utils

    S_pad, in_maps, j_sub, j_obj, offs = _host_prepare(
        hs, src_indices, tgt_perm, relationships, W_pred, b_pred)
    if _CACHE.get("S_pad") != S_pad:
        _CACHE["nc"] = _build_program(S_pad)
        _CACHE["S_pad"] = S_pad
    nc = _CACHE["nc"]

    res = bass_utils.run_bass_kernel_spmd(nc, in_maps, list(range(NCORES)))

    b = np.asarray(b_pred, dtype=np.float32)
    outs = []
    for c in range(NCORES):
        ab = res.results[c]["outab"][:P2].astype(np.float32)   # [102, S_pad]
        sl = slice(c * BLOC, (c + 1) * BLOC)
        col_sub = offs[c][:, :, None] + j_sub[:, sl]      # [L, BLOC, R]
        col_obj = offs[c][:, :, None] + j_obj[:, sl]
        logits = (ab[:P, col_sub] + ab[P:P2, col_obj])    # [P, L, BLOC, R]
        outs.append(np.ascontiguousarray(logits.transpose(1, 2, 3, 0) + b))
    return np.concatenate(outs, axis=1)


# revision 5
# speedup vs baseline: 1.1939x; 1.0599x over previous
"""DETR scene-graph predicate head on 8 Trainium2 NeuronCores.

Math: logits[l,b,r,:] = concat(hs[l,b,q_sub], hs[l,b,q_obj]) @ W_pred.T + b_pred
where q_sub/q_obj come from (tgt_perm inverse, relationships, src_indices) —
pure integer index math, done on host.

Key structure: relations only reference matched query slots, so only the
distinct queries actually used per (layer,image) block matter (~43 of 101 on
average).  The concat-linear decomposes per relation:
  logits[r,p] = A[q_sub(r),p] + B[q_obj(r),p] + b,  A = hs@W1.T, B = hs@W2.T
so the device computes the stacked A|B table (102 live channels) over the
ragged stream of used (block, query) slots with dense matmuls; the host does
the final O(L*B*R*P) index-select + add + bias.

Device layout (batch axis sharded 8 ways; L*B/8 = 192 blocks/core):
  - The per-core slot stream is fully ragged — no per-block padding.  Host
    concatenates each block's distinct-query hs columns into S_pad slots
    (padded only to the max core count, mult of 32) and splits the stream
    into T tiles of 512 slots (last ragged).  Block boundaries are irrelevant
    on device: every output column depends only on its own input column.
  - Per tile: hst cols [chunk0(W) | chunk1(W)] bf16 (d on partitions,
    2 chunks of 128 for the D=256 contraction), one ~256KB HWDGE load on the
    sync ring, 2 accumulating matmuls into one psum bank [128, W] f32 with
    the stationary wpk operand, one DVE/ACT cast (alternating) of the 102
    live rows to bf16, and one ~200KB store per tile pair on the scalar ring.
  - wpk [128, (chunk, 128)]: cols 0:51 W1.T, 51:102 W2.T, rest zero.

Streaming ~6MB/core through the 16 SDMA engines at ~350GB/s is the roofline;
compute (matmul ~7us, casts split across DVE+ACT) hides under it.
"""

import sys

import numpy as np

L, B, Q1, D = 6, 256, 101, 256
M, R, P = 64, 64, 51
NCORES = 8
BLOC = B // NCORES          # images per core
NB = L * BLOC               # (layer, image) blocks per core
TW = 512                    # slots per device tile (one psum bank)
P2 = 2 * P                  # 102 live logit channels (sub | obj halves)

_CACHE = {}


def _build_program(S_pad):
    import concourse.bacc as bacc
    import concourse.mybir as mybir
    import concourse.tile as tile
    from contextlib import ExitStack

    f32 = mybir.dt.float32
    bf16 = mybir.dt.bfloat16
    nc = bacc.Bacc("TRN2", target_bir_lowering=False, debug=False)

    T = -(-S_pad // TW)
    hst = nc.dram_tensor("hst", [128, 2 * S_pad], bf16, kind="ExternalInput").ap()
    wpk = nc.dram_tensor("wpk", [128, 256], bf16, kind="ExternalInput").ap()
    # 128 store rows (102 live + 26 zero): partition-balanced across the 16
    # SDMA engines — a 102-partition store lands on a subset of engines only.
    outab = nc.dram_tensor("outab", [128, S_pad], bf16, kind="ExternalOutput").ap()

    # input DMA groups (in tiles): small first group so the first matmul
    # starts early, then 1MB quads to amortize dispatch; stores per tile
    # pair on the SWDGE (gpsimd) queue so store dispatch never serializes
    # with the ACT casts on the scalar sequencer.
    groups = [1]
    rem = T - 1
    while rem > 0:
        g = min(4, rem)
        groups.append(g)
        rem -= g

    with tile.TileContext(nc) as tc, ExitStack() as ctx:
        const = ctx.enter_context(tc.tile_pool(name="const", bufs=1))
        inp = ctx.enter_context(tc.tile_pool(name="inp", bufs=3))
        outp = ctx.enter_context(tc.tile_pool(name="outp", bufs=3))
        psA = ctx.enter_context(tc.tile_pool(name="psA", bufs=6, space="PSUM"))

        wpk_t = const.tile([128, 256], bf16)
        nc.scalar.dma_start(out=wpk_t[:], in_=wpk[:])

        cast_flip = 0
        t = 0
        o_t = None
        for g in groups:
            t0 = t
            cols = 2 * (min(TW * (t0 + g), S_pad) - TW * t0)
            in_t = inp.tile([128, 8 * TW], bf16, tag="h")
            nc.sync.dma_start(out=in_t[:, 0:cols],
                              in_=hst[:, 2 * TW * t0:2 * TW * t0 + cols])

            for j in range(g):
                W = min(TW, S_pad - t * TW)
                lo = j * 2 * TW
                if t % 2 == 0:
                    o_t = outp.tile([128, 2 * TW], bf16, tag="o")
                half = (t % 2) * TW

                ps = psA.tile([128, TW], f32, tag="ps")
                nc.tensor.matmul(out=ps[:, 0:W], lhsT=wpk_t[:, 0:128],
                                 rhs=in_t[:, lo:lo + W], start=True, stop=False)
                nc.tensor.matmul(out=ps[:, 0:W], lhsT=wpk_t[:, 128:256],
                                 rhs=in_t[:, lo + W:lo + 2 * W],
                                 start=False, stop=True)
                if cast_flip == 0:
                    nc.vector.tensor_copy(out=o_t[:, half:half + W],
                                          in_=ps[:, 0:W])
                else:
                    nc.scalar.copy(out=o_t[:, half:half + W], in_=ps[:, 0:W])
                cast_flip ^= 1

                if t % 2 == 1 or t == T - 1:
                    p0 = (t // 2) * 2 * TW
                    n = min(2 * TW, S_pad - p0)
                    nc.gpsimd.dma_start(out=outab[:, p0:p0 + n],
                                        in_=o_t[:, 0:n])
                t += 1

    nc.compile()
    return nc


def _host_indices(src_indices, tgt_perm, relationships):
    """q_sub, q_obj: [L, B, R] int64 — query slot per relation."""
    src = np.asarray(src_indices, dtype=np.int64)
    tgt = np.asarray(tgt_perm, dtype=np.int64)
    rel = np.asarray(relationships, dtype=np.int64)

    # lookup[l, b, tgt[l, b, k]] = k
    lookup = np.empty((L, B, M), dtype=np.int64)
    li = np.arange(L)[:, None, None]
    bi = np.arange(B)[None, :, None]
    lookup[li, bi, tgt] = np.broadcast_to(np.arange(M), (L, B, M))

    sub_t = np.broadcast_to(rel[None, :, :, 0], (L, B, R))
    obj_t = np.broadcast_to(rel[None, :, :, 1], (L, B, R))
    pos_sub = np.take_along_axis(lookup, sub_t, axis=2)
    pos_obj = np.take_along_axis(lookup, obj_t, axis=2)
    q_sub = np.take_along_axis(src, pos_sub, axis=2)
    q_obj = np.take_along_axis(src, pos_obj, axis=2)
    return q_sub, q_obj


def _host_prepare(hs, src_indices, tgt_perm, relationships, W_pred, b_pred):
    """Ragged distinct-query compaction + device input packing."""
    import ml_dtypes
    bf16 = ml_dtypes.bfloat16

    hs = np.asarray(hs, dtype=np.float32)
    W = np.asarray(W_pred, dtype=np.float32)

    q_sub, q_obj = _host_indices(src_indices, tgt_perm, relationships)

    used = np.zeros((L * B, Q1), dtype=bool)
    rows = np.arange(L * B)[:, None]
    qcat = np.concatenate([q_sub, q_obj], axis=-1).reshape(L * B, 2 * R)
    used[rows, qcat] = True
    nuniq = used.sum(axis=1).reshape(L, B)
    # stable argsort of ~used: first nuniq entries = used queries, ascending
    order = np.argsort(~used, axis=1, kind="stable").reshape(L, B, Q1)
    slot_map = (np.cumsum(used, axis=1) - 1).reshape(L, B, Q1)
    j_sub = np.take_along_axis(slot_map, q_sub, axis=2)   # [L, B, R]
    j_obj = np.take_along_axis(slot_map, q_obj, axis=2)

    S_c = [int(nuniq[:, c * BLOC:(c + 1) * BLOC].sum()) for c in range(NCORES)]
    S_pad = -(-max(S_c) // 32) * 32
    T = -(-S_pad // TW)

    # wpk [128, (chunk, 128)]: chunk k cols = Wpad[128k:128k+128, :]
    wpad = np.zeros((D, 128), dtype=np.float32)
    wpad[:, :P] = W[:, :D].T
    wpad[:, P:P2] = W[:, D:].T
    wpk = np.ascontiguousarray(
        wpad.reshape(2, 128, 128).transpose(1, 0, 2).reshape(128, 256)
    ).astype(bf16)

    hs_bf = hs.astype(bf16)
    in_maps = []
    offs = []
    for c in range(NCORES):
        sl = slice(c * BLOC, (c + 1) * BLOC)
        n = nuniq[:, sl]                               # [L, BLOC]
        nf = n.reshape(-1)
        off = np.concatenate([[0], np.cumsum(nf)[:-1]]).reshape(L, BLOC)
        offs.append(off)

        mask = np.arange(Q1)[None, :] < nf[:, None]    # [NB, Q1]
        q_of = order[:, sl].reshape(NB, Q1)[mask]      # [S] ragged concat
        l_of = np.repeat(np.repeat(np.arange(L), BLOC), nf)
        b_of = np.repeat(np.tile(np.arange(c * BLOC, (c + 1) * BLOC), L), nf)
        pad = S_pad - q_of.shape[0]
        q_of = np.concatenate([q_of, np.zeros(pad, dtype=q_of.dtype)])
        l_of = np.concatenate([l_of, np.zeros(pad, dtype=l_of.dtype)])
        b_of = np.concatenate([b_of, np.zeros(pad, dtype=b_of.dtype)])

        G = hs_bf[l_of, b_of, q_of]                    # [S_pad, 256]
        Tm1 = T - 1
        main = (G[:TW * Tm1].reshape(Tm1, TW, 256).transpose(0, 2, 1)
                .reshape(Tm1, 2, 128, TW).transpose(2, 0, 1, 3)
                .reshape(128, Tm1 * 2 * TW))
        Wl = S_pad - TW * Tm1
        last = (G[TW * Tm1:].T.reshape(2, 128, Wl).transpose(1, 0, 2)
                .reshape(128, 2 * Wl))
        hst = np.ascontiguousarray(np.concatenate([main, last], axis=1))
        in_maps.append({"hst": hst, "wpk": wpk})
    return S_pad, in_maps, j_sub, j_obj, offs


def kernel(hs, src_indices, tgt_perm, relationships, W_pred, b_pred):
    if "concourse" not in sys.modules:
        try:
            import concourse  # noqa: F401
        except ImportError:
            sys.path.insert(0, "/opt/trn_rl_repo")
    from concourse import bass_utils

    S_pad, in_maps, j_sub, j_obj, offs = _host_prepare(
        hs, src_indices, tgt_perm, relationships, W_pred, b_pred)
    if _CACHE.get("S_pad") != S_pad:
        _CACHE["nc"] = _build_program(S_pad)
        _CACHE["S_pad"] = S_pad
    nc = _CACHE["nc"]

    res = bass_utils.run_bass_kernel_spmd(nc, in_maps, list(range(NCORES)))

    b = np.asarray(b_pred, dtype=np.float32)
    outs = []
    for c in range(NCORES):
        ab = res.results[c]["outab"][:P2].astype(np.float32)   # [102, S_pad]
        sl = slice(c * BLOC, (c + 1) * BLOC)
        col_sub = offs[c][:, :, None] + j_sub[:, sl]      # [L, BLOC, R]
        col_obj = offs[c][:, :, None] + j_obj[:, sl]
        logits = (ab[:P, col_sub] + ab[P:P2, col_obj])    # [P, L, BLOC, R]
        outs.append(np.ascontiguousarray(logits.transpose(1, 2, 3, 0) + b))
    return np.concatenate(outs, axis=1)
